# revision 24
# baseline (speedup 1.0000x reference)
"""Trainium2 Bass kernel for nn_Decoder_44770739094202.

Two-LSTM (merger + yielder) decoder with MDN heads, 50 recurrent steps.
Strategy:
  - Pure data parallel over 8 NeuronCores: batch 16384 -> 2048/core.
  - Per core, batch processed in 2 passes of 1024 columns (SBUF budget).
  - Transposed layout on device: everything is [feature, batch]; the host
    does all transposes / dtype casts so the device never transposes.
  - x_in = concat(enc_h, cond_t) where enc_h is constant across steps:
    the enc_h @ W part (+ gate bias b) is precomputed once per pass into
    DRAM scratch (bf16) and streamed back per step; only the rank-3 cond
    matmul and the recurrent h @ U matmul run inside the step loop.
  - Sigmoid synthesized as 0.5*tanh(x/2)+0.5 so that a single ACT table
    set (exp_and_others: exp/tanh/identity/copy) serves the whole kernel.
    The resulting *2 factors are folded into host-side weight scaling:
    U and Wh_h are pre-halved, the stored h state is 2*h.
  - MDN head bias (bh + cumsum of time_stamp rows of Wh) precomputed on
    host as a per-step [rows, 50] table; applied via ACT bias operand.
  - Softmax over the K=5 mixture rows (partition axis) via a ones[5,5]
    matmul (broadcast column sums), DVE reciprocal, DVE multiply.
"""

from contextlib import ExitStack

import numpy as np
import ml_dtypes

import concourse.bacc as bacc
import concourse.bass as bass
import concourse.tile as tile
from concourse import mybir
from concourse.bass_utils import run_bass_kernel_spmd

BF16 = ml_dtypes.bfloat16
F32 = np.float32

# Problem constants (hardcoded per contract).
B, S, D, K = 16384, 50, 512, 5
G4 = 4 * D          # 2048 gate width
NCORES = 8
BPC = B // NCORES   # 2048 batch per core
BT = 1024           # batch columns per pass
DC = D // 128       # 4 contraction chunks
GC = G4 // 128      # 16 gate row chunks
NB = 512            # matmul free-dim / psum bank width

AF = mybir.ActivationFunctionType
ALU = mybir.AluOpType


def build_program(n_steps=S, bt=BT):
    """Builds the per-core Bass program. Returns (nc, names)."""
    f32 = mybir.dt.float32
    bf16 = mybir.dt.bfloat16
    nb = bt // NB
    assert bt % NB == 0 and nb >= 1

    nc = bacc.Bacc()

    # ---- DRAM I/O ----
    enc_d = nc.dram_tensor("enc", [D, bt], bf16, kind="ExternalInput")
    h0_d = nc.dram_tensor("h0", [D, bt], bf16, kind="ExternalInput")
    c0_d = nc.dram_tensor("c0", [D, bt], f32, kind="ExternalInput")
    cond_d = nc.dram_tensor("cond", [3, n_steps, bt], bf16, kind="ExternalInput")
    um_d = nc.dram_tensor("um", [D, G4], bf16, kind="ExternalInput")
    uy_d = nc.dram_tensor("uy", [D, G4], bf16, kind="ExternalInput")
    wem_d = nc.dram_tensor("wem", [D, G4], bf16, kind="ExternalInput")
    wey_d = nc.dram_tensor("wey", [D, G4], bf16, kind="ExternalInput")
    wcm_d = nc.dram_tensor("wcm", [3, G4], bf16, kind="ExternalInput")
    wcy_d = nc.dram_tensor("wcy", [3, G4], bf16, kind="ExternalInput")
    # Head weights in padded device layout (32-aligned ACT groups):
    #   merger [512, 96]: [0:5]=a [5:10]=sl [10:15]=slt | [32:37]=r | [64:69]=ml [69:74]=mlt
    #   yielder [512, 64]: [0:5]=ay [5:10]=sly | [32:37]=mly
    whm_d = nc.dram_tensor("whm", [D, 96], bf16, kind="ExternalInput")
    why_d = nc.dram_tensor("why", [D, 64], bf16, kind="ExternalInput")
    bm_d = nc.dram_tensor("bm", [128, GC], f32, kind="ExternalInput")
    by_d = nc.dram_tensor("by", [128, GC], f32, kind="ExternalInput")
    btm_d = nc.dram_tensor("btm", [96, n_steps], f32, kind="ExternalInput")
    bty_d = nc.dram_tensor("bty", [64, n_steps], f32, kind="ExternalInput")
    # Device-order output rows (host permutes back to reference order):
    #   om: [0:5]=alpha [5:10]=exp(sl) [10:15]=exp(slt) [15:20]=tanh(r) [20:25]=ml [25:30]=mlt
    #   oy: [0:5]=alpha [5:10]=exp(sl) [10:15]=ml
    om_d = nc.dram_tensor("om", [n_steps, 30, bt], bf16, kind="ExternalOutput")
    oy_d = nc.dram_tensor("oy", [n_steps, 15, bt], bf16, kind="ExternalOutput")
    # DRAM scratch for precomputed enc@W + b (bf16), per LSTM.
    xwm_d = nc.dram_tensor("xwm", [GC, 128, bt], bf16, kind="Internal")
    xwy_d = nc.dram_tensor("xwy", [GC, 128, bt], bf16, kind="Internal")

    with tile.TileContext(nc) as tc, ExitStack() as ctx:
        wpool = ctx.enter_context(tc.tile_pool(name="wpool", bufs=1))
        spool = ctx.enter_context(tc.tile_pool(name="spool", bufs=1))
        prep = ctx.enter_context(tc.tile_pool(name="prep", bufs=1))
        hpool = ctx.enter_context(tc.tile_pool(name="hpool", bufs=2))
        xwin = ctx.enter_context(tc.tile_pool(name="xwin", bufs=4))
        zsbp = ctx.enter_context(tc.tile_pool(name="zsbp", bufs=3))
        tgp = ctx.enter_context(tc.tile_pool(name="tgp", bufs=2))
        ep = ctx.enter_context(tc.tile_pool(name="ep", bufs=2))
        omp = ctx.enter_context(tc.tile_pool(name="omp", bufs=2))
        zpsum = ctx.enter_context(tc.tile_pool(name="zpsum", bufs=2, space="PSUM"))
        hpsum = ctx.enter_context(tc.tile_pool(name="hpsum", bufs=2, space="PSUM"))
        spsum = ctx.enter_context(tc.tile_pool(name="spsum", bufs=2, space="PSUM"))

        # ---- weights into SBUF ----
        um_t = wpool.tile([128, DC, G4], bf16, tag="um")
        nc.sync.dma_start(out=um_t, in_=um_d[:].rearrange("(k p) g -> p k g", p=128))
        uy_t = wpool.tile([128, DC, G4], bf16, tag="uy")
        nc.sync.dma_start(out=uy_t, in_=uy_d[:].rearrange("(k p) g -> p k g", p=128))
        wcm_t = wpool.tile([3, G4], bf16, tag="wcm")
        nc.sync.dma_start(out=wcm_t, in_=wcm_d[:])
        wcy_t = wpool.tile([3, G4], bf16, tag="wcy")
        nc.sync.dma_start(out=wcy_t, in_=wcy_d[:])
        whm_t = wpool.tile([128, DC, 96], bf16, tag="whm")
        nc.sync.dma_start(out=whm_t, in_=whm_d[:].rearrange("(k p) g -> p k g", p=128))
        why_t = wpool.tile([128, DC, 64], bf16, tag="why")
        nc.sync.dma_start(out=why_t, in_=why_d[:].rearrange("(k p) g -> p k g", p=128))
        bm_t = wpool.tile([128, GC], f32, tag="bm")
        nc.sync.dma_start(out=bm_t, in_=bm_d[:])
        by_t = wpool.tile([128, GC], f32, tag="by")
        nc.sync.dma_start(out=by_t, in_=by_d[:])
        btm_t = wpool.tile([96, n_steps], f32, tag="btm")
        nc.sync.dma_start(out=btm_t, in_=btm_d[:])
        bty_t = wpool.tile([64, n_steps], f32, tag="bty")
        nc.sync.dma_start(out=bty_t, in_=bty_d[:])
        ones5 = wpool.tile([5, 5], bf16, tag="ones5")
        nc.vector.memset(ones5, 1.0)
        # Dependency-free ACT warmup so the one-time ACT table load attaches
        # here instead of consuming a sync slot on a real instruction.
        warm = wpool.tile([128, 8], f32, tag="warm")
        nc.vector.memset(warm, 0.0)
        nc.scalar.activation(warm, warm, AF.Tanh)
        nc.scalar.activation(warm, warm, AF.Exp)

        # ---- state into SBUF ----
        # encT shares the hTm tag: its slot is recycled once precompute ends.
        encT = hpool.tile([128, DC, bt], bf16, tag="hTm", name="encT")
        nc.sync.dma_start(out=encT, in_=enc_d[:].rearrange("(k p) b -> p k b", p=128))
        cT = {}
        hT = {}
        for L, (h0s, c0s) in {"m": (h0_d, c0_d), "y": (h0_d, c0_d)}.items():
            cT[L] = spool.tile([128, DC, bt], f32, tag=f"cT{L}", name=f"cT{L}")
            nc.sync.dma_start(
                out=cT[L], in_=c0s[:].rearrange("(k p) b -> p k b", p=128)
            )
            hT[L] = hpool.tile([128, DC, bt], bf16, tag=f"hT{L}", name=f"hT{L}0")
            nc.sync.dma_start(
                out=hT[L], in_=h0s[:].rearrange("(k p) b -> p k b", p=128)
            )

        # ---- precompute xW = enc @ W_e + b  ->  DRAM scratch (bf16) ----
        for L, (we_d, b_t, xw_d) in {
            "m": (wem_d, bm_t, xwm_d),
            "y": (wey_d, by_t, xwy_d),
        }.items():
            for m in range(GC):
                ps = zpsum.tile([128, bt], f32, tag="z")
                for k in range(DC):
                    we_km = prep.tile([128, 128], bf16, tag="wek", name="wekm")
                    nc.sync.dma_start(
                        out=we_km,
                        in_=we_d[:][
                            k * 128 : (k + 1) * 128, m * 128 : (m + 1) * 128
                        ],
                    )
                    for n in range(nb):
                        sl = slice(n * NB, (n + 1) * NB)
                        nc.tensor.matmul(
                            ps[:, sl],
                            we_km,
                            encT[:, k, sl],
                            start=(k == 0),
                            stop=(k == DC - 1),
                        )
                xz = zsbp.tile([128, bt], bf16, tag="xz")
                nc.scalar.activation(
                    xz, ps, AF.Identity, bias=b_t[:, m : m + 1], scale=1.0
                )
                nc.sync.dma_start(out=xw_d[:][m], in_=xz)

        # ---- recurrent steps ----
        # (act_func, psum/stage row range, bias rows) per head, 32-aligned.
        head_groups = {
            "m": [(AF.Exp, 0, 15), (AF.Tanh, 32, 37), (AF.Identity, 64, 74)],
            "y": [(AF.Exp, 0, 10), (AF.Identity, 32, 37)],
        }
        # (stage row range -> dram out row offset) DMA blocks per head.
        out_blocks = {
            "m": [(0, 15, 0), (32, 37, 15), (64, 74, 20)],
            "y": [(0, 10, 0), (32, 37, 10)],
        }
        hdims = {"m": 96, "y": 64}
        u_ts = {"m": um_t, "y": uy_t}
        wc_ts = {"m": wcm_t, "y": wcy_t}
        wh_ts = {"m": whm_t, "y": why_t}
        xw_ds = {"m": xwm_d, "y": xwy_d}
        btabs = {"m": btm_t, "y": bty_t}
        om_ds = {"m": om_d, "y": oy_d}

        for s in range(n_steps):
            condt = spool.tile([3, bt], bf16, tag="cond", bufs=4, name=f"cond{s}")
            nc.sync.dma_start(out=condt, in_=cond_d[:][:, s, :])
            for L in ("m", "y"):
                u_t, wc_t, wh_t = u_ts[L], wc_ts[L], wh_ts[L]
                xw_d, btab, om_dl = xw_ds[L], btabs[L], om_ds[L]
                hT_old = hT[L]
                hT_new = hpool.tile(
                    [128, DC, bt], mybir.dt.bfloat16, tag=f"hT{L}", name=f"hT{L}s{s}"
                )
                for dc in range(DC):
                    tg = {}
                    for g in range(4):
                        m = g * DC + dc
                        ps = zpsum.tile([128, bt], f32, tag="z")
                        for n in range(nb):
                            sl = slice(n * NB, (n + 1) * NB)
                            nc.tensor.matmul(
                                ps[:, sl],
                                wc_t[:, m * 128 : (m + 1) * 128],
                                condt[:, sl],
                                start=True,
                                stop=False,
                            )
                            for k in range(DC):
                                nc.tensor.matmul(
                                    ps[:, sl],
                                    u_t[:, k, m * 128 : (m + 1) * 128],
                                    hT_old[:, k, sl],
                                    start=False,
                                    stop=(k == DC - 1),
                                )
                        xz = xwin.tile([128, bt], mybir.dt.bfloat16, tag="xzin")
                        nc.sync.dma_start(out=xz, in_=xw_d[:][m])
                        zsb = zsbp.tile([128, bt], mybir.dt.bfloat16, tag="zsb")
                        nc.vector.tensor_add(zsb, ps, xz)
                        tgt = tgp.tile([128, bt], mybir.dt.bfloat16, tag=f"tg{g}")
                        nc.scalar.activation(
                            tgt, zsb, AF.Tanh, scale=(1.0 if g == 2 else 0.5)
                        )
                        tg[g] = tgt
                    # elementwise: c' = sig(f)c + sig(i)tanh(g); h' = sig(o)tanh(c')
                    A = ep.tile([128, bt], f32, tag="A")
                    nc.vector.scalar_tensor_tensor(
                        A, tg[1], 1.0, cT[L][:, dc, :], ALU.add, ALU.mult
                    )
                    si = ep.tile([128, bt], mybir.dt.bfloat16, tag="si")
                    nc.vector.tensor_scalar(
                        si, tg[0], 0.5, 0.5, ALU.mult, ALU.add
                    )
                    Bt = ep.tile([128, bt], mybir.dt.bfloat16, tag="Bt")
                    nc.vector.tensor_mul(Bt, si, tg[2])
                    nc.vector.scalar_tensor_tensor(
                        cT[L][:, dc, :], A, 0.5, Bt, ALU.mult, ALU.add
                    )
                    tc2 = ep.tile([128, bt], mybir.dt.bfloat16, tag="tc2")
                    nc.scalar.activation(tc2, cT[L][:, dc, :], AF.Tanh)
                    nc.vector.scalar_tensor_tensor(
                        hT_new[:, dc, :], tg[3], 1.0, tc2, ALU.add, ALU.mult
                    )
                hT[L] = hT_new
                # ---- MDN head ----
                hd = hdims[L]
                omt = omp.tile([hd, bt], mybir.dt.bfloat16, tag=f"om{L}")
                for n in range(nb):
                    sl = slice(n * NB, (n + 1) * NB)
                    hp = hpsum.tile([hd, NB], f32, tag="hp")
                    for k in range(DC):
                        nc.tensor.matmul(
                            hp,
                            wh_t[:, k, :],
                            hT_new[:, k, sl],
                            start=(k == 0),
                            stop=(k == DC - 1),
                        )
                    for (fn, a, b) in head_groups[L]:
                        nc.scalar.activation(
                            omt[a:b, sl], hp[a:b, :], fn,
                            bias=btab[a:b, s : s + 1],
                        )
                    sp = spsum.tile([5, NB], f32, tag="sp")
                    nc.tensor.matmul(sp, ones5, omt[0:5, sl], start=True, stop=True)
                    rc = ep.tile([5, NB], f32, tag="rc")
                    nc.vector.reciprocal(rc, sp)
                    nc.vector.tensor_mul(omt[0:5, sl], omt[0:5, sl], rc)
                for (a, b, o) in out_blocks[L]:
                    nc.sync.dma_start(
                        out=om_dl[:][s, o : o + (b - a), :], in_=omt[a:b, :]
                    )

    nc.compile()
    return nc


# Device head layout: (device col base, reference col slice) pairs.
#   merger ref cols: a=0:5 ml=5:10 sl=10:15 mlt=15:20 slt=20:25 r=25:30
HEAD_COLS_M = [(0, 0, 5), (5, 10, 15), (10, 20, 25), (32, 25, 30),
               (64, 5, 10), (69, 15, 20)]
#   yielder ref cols: a=0:5 ml=5:10 sl=10:15
HEAD_COLS_Y = [(0, 0, 5), (5, 10, 15), (32, 5, 10)]
# Device out rows -> reference out rows (om: [alpha, esl, eslt, tanh_r, ml, mlt])
PERM_M = np.array(
    list(range(0, 5)) + list(range(20, 25)) + list(range(5, 10))
    + list(range(25, 30)) + list(range(10, 15)) + list(range(15, 20))
)
PERM_Y = np.array(list(range(0, 5)) + list(range(10, 15)) + list(range(5, 10)))


def _pad_head(w, cols, width):
    """w [rows, ref_cols] -> padded [rows, width] in device column layout."""
    out = np.zeros((w.shape[0], width), w.dtype)
    for dev0, r0, r1 in cols:
        out[:, dev0 : dev0 + (r1 - r0)] = w[:, r0:r1]
    return out


def prep_host_inputs(inputs, n_steps=S):
    """Full-batch host-side prep. Returns dict of full-width arrays."""
    Wm = np.asarray(inputs["W_m"], F32)
    Wy = np.asarray(inputs["W_y"], F32)
    Um = np.asarray(inputs["U_m"], F32)
    Uy = np.asarray(inputs["U_y"], F32)
    Whm = np.asarray(inputs["Wh_m"], F32)
    Why = np.asarray(inputs["Wh_y"], F32)
    bm = np.asarray(inputs["b_m"], F32)
    by = np.asarray(inputs["b_y"], F32)
    bhm = np.asarray(inputs["bh_m"], F32)
    bhy = np.asarray(inputs["bh_y"], F32)
    sh = np.asarray(inputs["state_h"], F32)
    sc = np.asarray(inputs["state_c"], F32)
    cond = np.asarray(inputs["conditions"], F32)

    shT = np.ascontiguousarray(sh.T)                      # [512, B]
    out = {
        "um": np.ascontiguousarray((Um * 0.5).astype(BF16)),
        "uy": np.ascontiguousarray((Uy * 0.5).astype(BF16)),
        "wem": np.ascontiguousarray(Wm[:D].astype(BF16)),
        "wey": np.ascontiguousarray(Wy[:D].astype(BF16)),
        "wcm": np.ascontiguousarray(Wm[D:].astype(BF16)),
        "wcy": np.ascontiguousarray(Wy[D:].astype(BF16)),
        "whm": _pad_head(Whm[:D] * 0.5, HEAD_COLS_M, 96).astype(BF16),
        "why": _pad_head(Why[:D] * 0.5, HEAD_COLS_Y, 64).astype(BF16),
        "bm": np.ascontiguousarray(bm.reshape(GC, 128).T.astype(F32)),
        "by": np.ascontiguousarray(by.reshape(GC, 128).T.astype(F32)),
        "btm": _pad_head(
            (bhm[None, :] + np.cumsum(Whm[D : D + n_steps], 0)), HEAD_COLS_M, 96
        ).T.astype(F32).copy(),
        "bty": _pad_head(
            (bhy[None, :] + np.cumsum(Why[D : D + n_steps], 0)), HEAD_COLS_Y, 64
        ).T.astype(F32).copy(),
        "encT": np.ascontiguousarray(shT.astype(BF16)),   # [512, B]
        "h0T": np.ascontiguousarray((2.0 * shT).astype(BF16)),
        "c0T": np.ascontiguousarray(sc.T.astype(F32)),    # [512, B]
        "condT": np.ascontiguousarray(
            cond.transpose(2, 1, 0)[:, :n_steps, :].astype(BF16)
        ),                                                # [3, S, B]
    }
    return out


def make_in_maps(hp, col0, bt, n_cores=NCORES, stride=BPC):
    """One in_map per core for a pass covering columns [core*stride+col0, +bt)."""
    shared_keys = ["um", "uy", "wem", "wey", "wcm", "wcy", "whm", "why",
                   "bm", "by", "btm", "bty"]
    maps = []
    for ci in range(n_cores):
        a = ci * stride + col0
        sl = slice(a, a + bt)
        m = {k: hp[k] for k in shared_keys}
        m["enc"] = np.ascontiguousarray(hp["encT"][:, sl])
        m["h0"] = np.ascontiguousarray(hp["h0T"][:, sl])
        m["c0"] = np.ascontiguousarray(hp["c0T"][:, sl])
        m["cond"] = np.ascontiguousarray(hp["condT"][:, :, sl])
        maps.append(m)
    return maps


_PROG = {}


def _get_program(n_steps=S, bt=BT):
    key = (n_steps, bt)
    if key not in _PROG:
        _PROG[key] = build_program(n_steps, bt)
    return _PROG[key]


def kernel(**inputs):
    nc = _get_program()
    hp = prep_host_inputs(inputs)
    om_all = np.empty((S, 30, B), BF16)
    oy_all = np.empty((S, 15, B), BF16)
    for pi in range(BPC // BT):
        maps = make_in_maps(hp, pi * BT, BT)
        res = run_bass_kernel_spmd(nc, maps, core_ids=list(range(NCORES)))
        for ci in range(NCORES):
            a = ci * BPC + pi * BT
            sl = slice(a, a + BT)
            om_all[:, :, sl] = res.results[ci]["om"]
            oy_all[:, :, sl] = res.results[ci]["oy"]
    param_m = np.ascontiguousarray(
        om_all.transpose(2, 0, 1)[:, :, PERM_M].astype(F32)
    )
    param_y = np.ascontiguousarray(
        oy_all.transpose(2, 0, 1)[:, :, PERM_Y].astype(F32)
    )
    return param_m, param_y


# revision 25
# speedup vs baseline: 25.6325x; 25.6325x over previous
"""Trainium2 Bass kernel for nn_Decoder_44770739094202.

Two-LSTM (merger + yielder) decoder with MDN heads, 50 recurrent steps.
Strategy:
  - Pure data parallel over 8 NeuronCores: batch 16384 -> 2048/core.
  - Per core, batch processed in 2 passes of 1024 columns (SBUF budget).
  - Transposed layout on device: everything is [feature, batch]; the host
    does all transposes / dtype casts so the device never transposes.
  - x_in = concat(enc_h, cond_t) where enc_h is constant across steps:
    the enc_h @ W part (+ gate bias b) is precomputed once per pass into
    DRAM scratch (bf16) and streamed back per step; only the rank-3 cond
    matmul and the recurrent h @ U matmul run inside the step loop.
  - Sigmoid synthesized as 0.5*tanh(x/2)+0.5 so that a single ACT table
    set (exp_and_others: exp/tanh/identity/copy) serves the whole kernel.
    The resulting *2 factors are folded into host-side weight scaling:
    U and Wh_h are pre-halved, the stored h state is 2*h.
  - MDN head bias (bh + cumsum of time_stamp rows of Wh) precomputed on
    host as a per-step [rows, 50] table; applied via ACT bias operand.
  - Softmax over the K=5 mixture rows (partition axis) via a ones[5,5]
    matmul (broadcast column sums), DVE reciprocal, DVE multiply.
"""

from contextlib import ExitStack

import numpy as np
import ml_dtypes

import concourse.bacc as bacc
import concourse.tile as tile
from concourse import mybir
from concourse.bass_utils import run_bass_kernel_spmd

BF16 = ml_dtypes.bfloat16
F32 = np.float32

# Problem constants (hardcoded per contract).
B, S, D, K = 16384, 50, 512, 5
G4 = 4 * D          # 2048 gate width
NCORES = 8
BPC = B // NCORES   # 2048 batch per core
BT = 1024           # batch columns per pass
DC = D // 128       # 4 contraction chunks
GC = G4 // 128      # 16 gate row chunks
NB = 512            # matmul free-dim / psum bank width

AF = mybir.ActivationFunctionType
ALU = mybir.AluOpType


def build_program(n_steps=S, bt=BT):
    """Builds the per-core Bass program. Returns (nc, names)."""
    f32 = mybir.dt.float32
    bf16 = mybir.dt.bfloat16
    nb = bt // NB
    assert bt % NB == 0 and nb >= 1

    nc = bacc.Bacc()

    # ---- DRAM I/O ----
    enc_d = nc.dram_tensor("enc", [D, bt], bf16, kind="ExternalInput")
    h0_d = nc.dram_tensor("h0", [D, bt], bf16, kind="ExternalInput")
    c0_d = nc.dram_tensor("c0", [D, bt], f32, kind="ExternalInput")
    cond_d = nc.dram_tensor("cond", [3, n_steps, bt], bf16, kind="ExternalInput")
    um_d = nc.dram_tensor("um", [D, G4], bf16, kind="ExternalInput")
    uy_d = nc.dram_tensor("uy", [D, G4], bf16, kind="ExternalInput")
    wem_d = nc.dram_tensor("wem", [D, G4], bf16, kind="ExternalInput")
    wey_d = nc.dram_tensor("wey", [D, G4], bf16, kind="ExternalInput")
    wcm_d = nc.dram_tensor("wcm", [3, G4], bf16, kind="ExternalInput")
    wcy_d = nc.dram_tensor("wcy", [3, G4], bf16, kind="ExternalInput")
    # Head weights in padded device layout (32-aligned ACT groups):
    #   merger [512, 96]: [0:5]=a [5:10]=sl [10:15]=slt | [32:37]=r | [64:69]=ml [69:74]=mlt
    #   yielder [512, 64]: [0:5]=ay [5:10]=sly | [32:37]=mly
    whm_d = nc.dram_tensor("whm", [D, 96], bf16, kind="ExternalInput")
    why_d = nc.dram_tensor("why", [D, 64], bf16, kind="ExternalInput")
    bm_d = nc.dram_tensor("bm", [128, GC], f32, kind="ExternalInput")
    by_d = nc.dram_tensor("by", [128, GC], f32, kind="ExternalInput")
    btm_d = nc.dram_tensor("btm", [96, n_steps], f32, kind="ExternalInput")
    bty_d = nc.dram_tensor("bty", [64, n_steps], f32, kind="ExternalInput")
    # Device-order output rows (host permutes back to reference order):
    #   om: [0:5]=alpha [5:10]=exp(sl) [10:15]=exp(slt) [15:20]=tanh(r) [20:25]=ml [25:30]=mlt
    #   oy: [0:5]=alpha [5:10]=exp(sl) [10:15]=ml
    om_d = nc.dram_tensor("om", [n_steps, 30, bt], bf16, kind="ExternalOutput")
    oy_d = nc.dram_tensor("oy", [n_steps, 15, bt], bf16, kind="ExternalOutput")
    # DRAM scratch for precomputed enc@W + b (bf16), per LSTM.
    xwm_d = nc.dram_tensor("xwm", [GC, 128, bt], bf16, kind="Internal")
    xwy_d = nc.dram_tensor("xwy", [GC, 128, bt], bf16, kind="Internal")

    with tile.TileContext(nc) as tc, ExitStack() as ctx:
        wpool = ctx.enter_context(tc.tile_pool(name="wpool", bufs=1))
        spool = ctx.enter_context(tc.tile_pool(name="spool", bufs=1))
        prep = ctx.enter_context(tc.tile_pool(name="prep", bufs=1))
        hpool = ctx.enter_context(tc.tile_pool(name="hpool", bufs=2))
        xwin = ctx.enter_context(tc.tile_pool(name="xwin", bufs=4))
        zsbp = ctx.enter_context(tc.tile_pool(name="zsbp", bufs=3))
        tgp = ctx.enter_context(tc.tile_pool(name="tgp", bufs=2))
        ep = ctx.enter_context(tc.tile_pool(name="ep", bufs=2))
        omp = ctx.enter_context(tc.tile_pool(name="omp", bufs=2))
        zpsum = ctx.enter_context(tc.tile_pool(name="zpsum", bufs=2, space="PSUM"))
        hpsum = ctx.enter_context(tc.tile_pool(name="hpsum", bufs=2, space="PSUM"))
        spsum = ctx.enter_context(tc.tile_pool(name="spsum", bufs=2, space="PSUM"))

        # ---- weights into SBUF ----
        um_t = wpool.tile([128, DC, G4], bf16, tag="um")
        nc.sync.dma_start(out=um_t, in_=um_d[:].rearrange("(k p) g -> p k g", p=128))
        uy_t = wpool.tile([128, DC, G4], bf16, tag="uy")
        nc.sync.dma_start(out=uy_t, in_=uy_d[:].rearrange("(k p) g -> p k g", p=128))
        wcm_t = wpool.tile([3, G4], bf16, tag="wcm")
        nc.sync.dma_start(out=wcm_t, in_=wcm_d[:])
        wcy_t = wpool.tile([3, G4], bf16, tag="wcy")
        nc.sync.dma_start(out=wcy_t, in_=wcy_d[:])
        whm_t = wpool.tile([128, DC, 96], bf16, tag="whm")
        nc.sync.dma_start(out=whm_t, in_=whm_d[:].rearrange("(k p) g -> p k g", p=128))
        why_t = wpool.tile([128, DC, 64], bf16, tag="why")
        nc.sync.dma_start(out=why_t, in_=why_d[:].rearrange("(k p) g -> p k g", p=128))
        bm_t = wpool.tile([128, GC], f32, tag="bm")
        nc.sync.dma_start(out=bm_t, in_=bm_d[:])
        by_t = wpool.tile([128, GC], f32, tag="by")
        nc.sync.dma_start(out=by_t, in_=by_d[:])
        btm_t = wpool.tile([96, n_steps], f32, tag="btm")
        nc.sync.dma_start(out=btm_t, in_=btm_d[:])
        bty_t = wpool.tile([64, n_steps], f32, tag="bty")
        nc.sync.dma_start(out=bty_t, in_=bty_d[:])
        ones5 = wpool.tile([5, 5], bf16, tag="ones5")
        nc.vector.memset(ones5, 1.0)
        # Dependency-free ACT warmup so the one-time ACT table load attaches
        # here instead of consuming a sync slot on a real instruction.
        warm = wpool.tile([128, 8], f32, tag="warm")
        nc.vector.memset(warm, 0.0)
        nc.scalar.activation(warm, warm, AF.Tanh)
        nc.scalar.activation(warm, warm, AF.Exp)

        # ---- state into SBUF ----
        # encT shares the hTm tag: its slot is recycled once precompute ends.
        encT = hpool.tile([128, DC, bt], bf16, tag="hTm", name="encT")
        nc.sync.dma_start(out=encT, in_=enc_d[:].rearrange("(k p) b -> p k b", p=128))
        cT = {}
        hT = {}
        for L, (h0s, c0s) in {"m": (h0_d, c0_d), "y": (h0_d, c0_d)}.items():
            cT[L] = spool.tile([128, DC, bt], f32, tag=f"cT{L}", name=f"cT{L}")
            nc.sync.dma_start(
                out=cT[L], in_=c0s[:].rearrange("(k p) b -> p k b", p=128)
            )
            hT[L] = hpool.tile([128, DC, bt], bf16, tag=f"hT{L}", name=f"hT{L}0")
            nc.sync.dma_start(
                out=hT[L], in_=h0s[:].rearrange("(k p) b -> p k b", p=128)
            )

        # ---- precompute xW = enc @ W_e + b  ->  DRAM scratch (bf16) ----
        for L, (we_d, b_t, xw_d) in {
            "m": (wem_d, bm_t, xwm_d),
            "y": (wey_d, by_t, xwy_d),
        }.items():
            for m in range(GC):
                ps = zpsum.tile([128, bt], f32, tag="z")
                for k in range(DC):
                    we_km = prep.tile([128, 128], bf16, tag="wek", name="wekm")
                    nc.sync.dma_start(
                        out=we_km,
                        in_=we_d[:][
                            k * 128 : (k + 1) * 128, m * 128 : (m + 1) * 128
                        ],
                    )
                    for n in range(nb):
                        sl = slice(n * NB, (n + 1) * NB)
                        nc.tensor.matmul(
                            ps[:, sl],
                            we_km,
                            encT[:, k, sl],
                            start=(k == 0),
                            stop=(k == DC - 1),
                        )
                xz = zsbp.tile([128, bt], bf16, tag="xz")
                nc.scalar.activation(
                    xz, ps, AF.Identity, bias=b_t[:, m : m + 1], scale=1.0
                )
                nc.sync.dma_start(out=xw_d[:][m], in_=xz)

        # ---- recurrent steps ----
        # (act_func, psum/stage row range, bias rows) per head, 32-aligned.
        head_groups = {
            "m": [(AF.Exp, 0, 15), (AF.Tanh, 32, 37), (AF.Identity, 64, 74)],
            "y": [(AF.Exp, 0, 10), (AF.Identity, 32, 37)],
        }
        # (stage row range -> dram out row offset) DMA blocks per head.
        out_blocks = {
            "m": [(0, 15, 0), (32, 37, 15), (64, 74, 20)],
            "y": [(0, 10, 0), (32, 37, 10)],
        }
        hdims = {"m": 96, "y": 64}
        u_ts = {"m": um_t, "y": uy_t}
        wc_ts = {"m": wcm_t, "y": wcy_t}
        wh_ts = {"m": whm_t, "y": why_t}
        xw_ds = {"m": xwm_d, "y": xwy_d}
        btabs = {"m": btm_t, "y": bty_t}
        om_ds = {"m": om_d, "y": oy_d}

        for s in range(n_steps):
            condt = spool.tile([3, bt], bf16, tag="cond", bufs=4, name=f"cond{s}")
            nc.sync.dma_start(out=condt, in_=cond_d[:][:, s, :])
            for L in ("m", "y"):
                u_t, wc_t, wh_t = u_ts[L], wc_ts[L], wh_ts[L]
                xw_d, btab, om_dl = xw_ds[L], btabs[L], om_ds[L]
                hT_old = hT[L]
                hT_new = hpool.tile(
                    [128, DC, bt], mybir.dt.bfloat16, tag=f"hT{L}", name=f"hT{L}s{s}"
                )
                for dc in range(DC):
                    tg = {}
                    for g in range(4):
                        m = g * DC + dc
                        ps = zpsum.tile([128, bt], f32, tag="z")
                        for n in range(nb):
                            sl = slice(n * NB, (n + 1) * NB)
                            nc.tensor.matmul(
                                ps[:, sl],
                                wc_t[:, m * 128 : (m + 1) * 128],
                                condt[:, sl],
                                start=True,
                                stop=False,
                            )
                            for k in range(DC):
                                nc.tensor.matmul(
                                    ps[:, sl],
                                    u_t[:, k, m * 128 : (m + 1) * 128],
                                    hT_old[:, k, sl],
                                    start=False,
                                    stop=(k == DC - 1),
                                )
                        xz = xwin.tile([128, bt], mybir.dt.bfloat16, tag="xzin")
                        nc.sync.dma_start(out=xz, in_=xw_d[:][m])
                        zsb = zsbp.tile([128, bt], mybir.dt.bfloat16, tag="zsb")
                        nc.vector.tensor_add(zsb, ps, xz)
                        tgt = tgp.tile([128, bt], mybir.dt.bfloat16, tag=f"tg{g}")
                        nc.scalar.activation(
                            tgt, zsb, AF.Tanh, scale=(1.0 if g == 2 else 0.5)
                        )
                        tg[g] = tgt
                    # elementwise: c' = sig(f)c + sig(i)tanh(g); h' = sig(o)tanh(c')
                    A = ep.tile([128, bt], f32, tag="A")
                    nc.vector.scalar_tensor_tensor(
                        A, tg[1], 1.0, cT[L][:, dc, :], ALU.add, ALU.mult
                    )
                    si = ep.tile([128, bt], mybir.dt.bfloat16, tag="si")
                    nc.vector.tensor_scalar(
                        si, tg[0], 0.5, 0.5, ALU.mult, ALU.add
                    )
                    Bt = ep.tile([128, bt], mybir.dt.bfloat16, tag="Bt")
                    nc.vector.tensor_mul(Bt, si, tg[2])
                    nc.vector.scalar_tensor_tensor(
                        cT[L][:, dc, :], A, 0.5, Bt, ALU.mult, ALU.add
                    )
                    tc2 = ep.tile([128, bt], mybir.dt.bfloat16, tag="tc2")
                    nc.scalar.activation(tc2, cT[L][:, dc, :], AF.Tanh)
                    nc.vector.scalar_tensor_tensor(
                        hT_new[:, dc, :], tg[3], 1.0, tc2, ALU.add, ALU.mult
                    )
                hT[L] = hT_new
                # ---- MDN head ----
                hd = hdims[L]
                omt = omp.tile([hd, bt], mybir.dt.bfloat16, tag=f"om{L}")
                for n in range(nb):
                    sl = slice(n * NB, (n + 1) * NB)
                    hp = hpsum.tile([hd, NB], f32, tag="hp")
                    for k in range(DC):
                        nc.tensor.matmul(
                            hp,
                            wh_t[:, k, :],
                            hT_new[:, k, sl],
                            start=(k == 0),
                            stop=(k == DC - 1),
                        )
                    for (fn, a, b) in head_groups[L]:
                        nc.scalar.activation(
                            omt[a:b, sl], hp[a:b, :], fn,
                            bias=btab[a:b, s : s + 1],
                        )
                    sp = spsum.tile([5, NB], f32, tag="sp")
                    nc.tensor.matmul(sp, ones5, omt[0:5, sl], start=True, stop=True)
                    rc = ep.tile([5, NB], f32, tag="rc")
                    nc.vector.reciprocal(rc, sp)
                    nc.vector.tensor_mul(omt[0:5, sl], omt[0:5, sl], rc)
                for (a, b, o) in out_blocks[L]:
                    nc.sync.dma_start(
                        out=om_dl[:][s, o : o + (b - a), :], in_=omt[a:b, :]
                    )

    nc.compile()
    return nc


# Device head layout: (device col base, reference col slice) pairs.
#   merger ref cols: a=0:5 ml=5:10 sl=10:15 mlt=15:20 slt=20:25 r=25:30
HEAD_COLS_M = [(0, 0, 5), (5, 10, 15), (10, 20, 25), (32, 25, 30),
               (64, 5, 10), (69, 15, 20)]
#   yielder ref cols: a=0:5 ml=5:10 sl=10:15
HEAD_COLS_Y = [(0, 0, 5), (5, 10, 15), (32, 5, 10)]
# Device out rows -> reference out rows (om: [alpha, esl, eslt, tanh_r, ml, mlt])
PERM_M = np.array(
    list(range(0, 5)) + list(range(20, 25)) + list(range(5, 10))
    + list(range(25, 30)) + list(range(10, 15)) + list(range(15, 20))
)
PERM_Y = np.array(list(range(0, 5)) + list(range(10, 15)) + list(range(5, 10)))


def _pad_head(w, cols, width):
    """w [rows, ref_cols] -> padded [rows, width] in device column layout."""
    out = np.zeros((w.shape[0], width), w.dtype)
    for dev0, r0, r1 in cols:
        out[:, dev0 : dev0 + (r1 - r0)] = w[:, r0:r1]
    return out


def prep_host_inputs(inputs, n_steps=S):
    """Full-batch host-side prep. Returns dict of full-width arrays."""
    Wm = np.asarray(inputs["W_m"], F32)
    Wy = np.asarray(inputs["W_y"], F32)
    Um = np.asarray(inputs["U_m"], F32)
    Uy = np.asarray(inputs["U_y"], F32)
    Whm = np.asarray(inputs["Wh_m"], F32)
    Why = np.asarray(inputs["Wh_y"], F32)
    bm = np.asarray(inputs["b_m"], F32)
    by = np.asarray(inputs["b_y"], F32)
    bhm = np.asarray(inputs["bh_m"], F32)
    bhy = np.asarray(inputs["bh_y"], F32)
    sh = np.asarray(inputs["state_h"], F32)
    sc = np.asarray(inputs["state_c"], F32)
    cond = np.asarray(inputs["conditions"], F32)

    shT = np.ascontiguousarray(sh.T)                      # [512, B]
    out = {
        "um": np.ascontiguousarray((Um * 0.5).astype(BF16)),
        "uy": np.ascontiguousarray((Uy * 0.5).astype(BF16)),
        "wem": np.ascontiguousarray(Wm[:D].astype(BF16)),
        "wey": np.ascontiguousarray(Wy[:D].astype(BF16)),
        "wcm": np.ascontiguousarray(Wm[D:].astype(BF16)),
        "wcy": np.ascontiguousarray(Wy[D:].astype(BF16)),
        "whm": _pad_head(Whm[:D] * 0.5, HEAD_COLS_M, 96).astype(BF16),
        "why": _pad_head(Why[:D] * 0.5, HEAD_COLS_Y, 64).astype(BF16),
        "bm": np.ascontiguousarray(bm.reshape(GC, 128).T.astype(F32)),
        "by": np.ascontiguousarray(by.reshape(GC, 128).T.astype(F32)),
        "btm": _pad_head(
            (bhm[None, :] + np.cumsum(Whm[D : D + n_steps], 0)), HEAD_COLS_M, 96
        ).T.astype(F32).copy(),
        "bty": _pad_head(
            (bhy[None, :] + np.cumsum(Why[D : D + n_steps], 0)), HEAD_COLS_Y, 64
        ).T.astype(F32).copy(),
        "encT": np.ascontiguousarray(shT.astype(BF16)),   # [512, B]
        "h0T": np.ascontiguousarray((2.0 * shT).astype(BF16)),
        "c0T": np.ascontiguousarray(sc.T.astype(F32)),    # [512, B]
        "condT": np.ascontiguousarray(
            cond.transpose(2, 1, 0)[:, :n_steps, :].astype(BF16)
        ),                                                # [3, S, B]
    }
    return out


def make_in_maps(hp, col0, bt, n_cores=NCORES, stride=BPC):
    """One in_map per core for a pass covering columns [core*stride+col0, +bt)."""
    shared_keys = ["um", "uy", "wem", "wey", "wcm", "wcy", "whm", "why",
                   "bm", "by", "btm", "bty"]
    maps = []
    for ci in range(n_cores):
        a = ci * stride + col0
        sl = slice(a, a + bt)
        m = {k: hp[k] for k in shared_keys}
        m["enc"] = np.ascontiguousarray(hp["encT"][:, sl])
        m["h0"] = np.ascontiguousarray(hp["h0T"][:, sl])
        m["c0"] = np.ascontiguousarray(hp["c0T"][:, sl])
        m["cond"] = np.ascontiguousarray(hp["condT"][:, :, sl])
        maps.append(m)
    return maps


_PROG = {}


def _get_program(n_steps=S, bt=BT):
    key = (n_steps, bt)
    if key not in _PROG:
        _PROG[key] = build_program(n_steps, bt)
    return _PROG[key]


def kernel(**inputs):
    nc = _get_program()
    hp = prep_host_inputs(inputs)
    om_all = np.empty((S, 30, B), BF16)
    oy_all = np.empty((S, 15, B), BF16)
    for pi in range(BPC // BT):
        maps = make_in_maps(hp, pi * BT, BT)
        res = run_bass_kernel_spmd(nc, maps, core_ids=list(range(NCORES)))
        for ci in range(NCORES):
            a = ci * BPC + pi * BT
            sl = slice(a, a + BT)
            om_all[:, :, sl] = res.results[ci]["om"]
            oy_all[:, :, sl] = res.results[ci]["oy"]
    param_m = np.ascontiguousarray(
        om_all.transpose(2, 0, 1)[:, :, PERM_M].astype(F32)
    )
    param_y = np.ascontiguousarray(
        oy_all.transpose(2, 0, 1)[:, :, PERM_Y].astype(F32)
    )
    return param_m, param_y


# revision 44
# speedup vs baseline: 28.9114x; 1.1279x over previous
"""Trainium2 Bass kernel for nn_Decoder_44770739094202.

Two-LSTM (merger + yielder) decoder with MDN heads, 50 recurrent steps.
Strategy:
  - Pure data parallel over 8 NeuronCores: batch 16384 -> 2048/core.
  - Per core, batch processed in 2 passes of 1024 columns (SBUF budget).
  - Transposed layout on device: everything is [feature, batch]; the host
    does all transposes / dtype casts so the device never transposes.
  - x_in = concat(enc_h, cond_t) where enc_h is constant across steps:
    the enc_h @ W part (+ gate bias b) is precomputed once per pass into
    DRAM scratch (bf16) and streamed back per step; only the rank-3 cond
    matmul and the recurrent h @ U matmul run inside the step loop.
  - Sigmoid synthesized as 0.5*tanh(x/2)+0.5 so that a single ACT table
    set (exp_and_others: exp/tanh/identity/copy) serves the whole kernel.
    The resulting *2 factors are folded into host-side weight scaling:
    U and Wh_h are pre-halved, the stored h state is 2*h.
  - MDN head bias (bh + cumsum of time_stamp rows of Wh) precomputed on
    host as a per-step [rows, 50] table; applied via ACT bias operand.
  - Softmax over the K=5 mixture rows (partition axis) via a ones[5,5]
    matmul (broadcast column sums), DVE reciprocal, DVE multiply.
"""

from contextlib import ExitStack

import numpy as np
import ml_dtypes

import concourse.bacc as bacc
import concourse.tile as tile
from concourse import mybir
from concourse.bass_utils import run_bass_kernel_spmd

BF16 = ml_dtypes.bfloat16
F32 = np.float32

# Problem constants (hardcoded per contract).
B, S, D, K = 16384, 50, 512, 5
G4 = 4 * D          # 2048 gate width
NCORES = 8
BPC = B // NCORES   # 2048 batch per core
BT = 1024           # batch columns per pass
DC = D // 128       # 4 contraction chunks
GC = G4 // 128      # 16 gate row chunks
NB = 512            # matmul free-dim / psum bank width

AF = mybir.ActivationFunctionType
ALU = mybir.AluOpType


def build_program(n_steps=S, bt=BT, ablate=frozenset()):
    """Builds the per-core Bass program. Returns (nc, names).

    ablate: cost-model ablation tokens ("zadd", "cond", "elem", "heads") —
    produce numerically wrong but structurally comparable programs for
    TimelineSim A/B attribution. Never used by kernel().
    """
    f32 = mybir.dt.float32
    bf16 = mybir.dt.bfloat16
    nb = bt // NB
    assert bt % NB == 0 and nb >= 1

    nc = bacc.Bacc()

    # ---- DRAM I/O ----
    enc_d = nc.dram_tensor("enc", [D, bt], bf16, kind="ExternalInput")
    h0_d = nc.dram_tensor("h0", [D, bt], bf16, kind="ExternalInput")
    c0_d = nc.dram_tensor("c0", [D, bt], f32, kind="ExternalInput")
    cond_d = nc.dram_tensor("cond", [3, n_steps, bt], bf16, kind="ExternalInput")
    um_d = nc.dram_tensor("um", [D, G4], bf16, kind="ExternalInput")
    uy_d = nc.dram_tensor("uy", [D, G4], bf16, kind="ExternalInput")
    wem_d = nc.dram_tensor("wem", [D, G4], bf16, kind="ExternalInput")
    wey_d = nc.dram_tensor("wey", [D, G4], bf16, kind="ExternalInput")
    wcm_d = nc.dram_tensor("wcm", [3, G4], bf16, kind="ExternalInput")
    wcy_d = nc.dram_tensor("wcy", [3, G4], bf16, kind="ExternalInput")
    # Head weights in padded device layout (32-aligned ACT groups):
    #   merger [512, 96]: [0:5]=a [5:10]=sl [10:15]=slt | [32:37]=r | [64:69]=ml [69:74]=mlt
    #   yielder [512, 64]: [0:5]=ay [5:10]=sly | [32:37]=mly
    whm_d = nc.dram_tensor("whm", [D, 96], bf16, kind="ExternalInput")
    why_d = nc.dram_tensor("why", [D, 64], bf16, kind="ExternalInput")
    bm_d = nc.dram_tensor("bm", [128, GC], f32, kind="ExternalInput")
    by_d = nc.dram_tensor("by", [128, GC], f32, kind="ExternalInput")
    btm_d = nc.dram_tensor("btm", [96, n_steps], f32, kind="ExternalInput")
    bty_d = nc.dram_tensor("bty", [64, n_steps], f32, kind="ExternalInput")
    # Device-order output rows (host permutes back to reference order):
    #   om: [0:5]=alpha [5:10]=exp(sl) [10:15]=exp(slt) [15:20]=tanh(r) [20:25]=ml [25:30]=mlt
    #   oy: [0:5]=alpha [5:10]=exp(sl) [10:15]=ml
    om_d = nc.dram_tensor("om", [n_steps, 30, bt], bf16, kind="ExternalOutput")
    oy_d = nc.dram_tensor("oy", [n_steps, 15, bt], bf16, kind="ExternalOutput")
    # DRAM scratch for precomputed enc@W + b (bf16), per LSTM.
    xwm_d = nc.dram_tensor("xwm", [GC, 128, bt], bf16, kind="Internal")
    xwy_d = nc.dram_tensor("xwy", [GC, 128, bt], bf16, kind="Internal")

    with tile.TileContext(nc) as tc, ExitStack() as ctx:
        wpool = ctx.enter_context(tc.tile_pool(name="wpool", bufs=1))
        spool = ctx.enter_context(tc.tile_pool(name="spool", bufs=1))
        prep = ctx.enter_context(tc.tile_pool(name="prep", bufs=16))
        hpool = ctx.enter_context(tc.tile_pool(name="hpool", bufs=2))
        xwin = ctx.enter_context(tc.tile_pool(name="xwin", bufs=4))
        zsbp = ctx.enter_context(tc.tile_pool(name="zsbp", bufs=3))
        tgp = ctx.enter_context(tc.tile_pool(name="tgp", bufs=2))
        ep = ctx.enter_context(tc.tile_pool(name="ep", bufs=2))
        omp = ctx.enter_context(tc.tile_pool(name="omp", bufs=2))
        ps3 = "ps2" not in ablate
        zpsum = ctx.enter_context(
            tc.tile_pool(name="zpsum", bufs=(3 if ps3 else 2), space="PSUM")
        )
        hpsum = ctx.enter_context(
            tc.tile_pool(name="hpsum", bufs=(1 if ps3 else 2), space="PSUM")
        )
        spsum = ctx.enter_context(
            tc.tile_pool(name="spsum", bufs=(1 if ps3 else 2), space="PSUM")
        )

        # ---- weights into SBUF ----
        um_t = wpool.tile([128, DC, G4], bf16, tag="um")
        nc.sync.dma_start(out=um_t, in_=um_d[:].rearrange("(k p) g -> p k g", p=128))
        uy_t = wpool.tile([128, DC, G4], bf16, tag="uy")
        nc.sync.dma_start(out=uy_t, in_=uy_d[:].rearrange("(k p) g -> p k g", p=128))
        wcm_t = wpool.tile([3, G4], bf16, tag="wcm")
        nc.sync.dma_start(out=wcm_t, in_=wcm_d[:])
        wcy_t = wpool.tile([3, G4], bf16, tag="wcy")
        nc.sync.dma_start(out=wcy_t, in_=wcy_d[:])
        whm_t = wpool.tile([128, DC, 96], bf16, tag="whm")
        nc.sync.dma_start(out=whm_t, in_=whm_d[:].rearrange("(k p) g -> p k g", p=128))
        why_t = wpool.tile([128, DC, 64], bf16, tag="why")
        nc.sync.dma_start(out=why_t, in_=why_d[:].rearrange("(k p) g -> p k g", p=128))
        bm_t = wpool.tile([128, GC], f32, tag="bm")
        nc.sync.dma_start(out=bm_t, in_=bm_d[:])
        by_t = wpool.tile([128, GC], f32, tag="by")
        nc.sync.dma_start(out=by_t, in_=by_d[:])
        btm_t = wpool.tile([96, n_steps], f32, tag="btm")
        nc.sync.dma_start(out=btm_t, in_=btm_d[:])
        bty_t = wpool.tile([64, n_steps], f32, tag="bty")
        nc.sync.dma_start(out=bty_t, in_=bty_d[:])
        ones5 = wpool.tile([5, 5], bf16, tag="ones5")
        nc.vector.memset(ones5, 1.0)
        # Dependency-free ACT warmup so the one-time ACT table load attaches
        # here instead of consuming a sync slot on a real instruction.
        warm = wpool.tile([128, 8], f32, tag="warm")
        nc.vector.memset(warm, 0.0)
        nc.scalar.activation(warm, warm, AF.Tanh)
        nc.scalar.activation(warm, warm, AF.Exp)

        # ---- state into SBUF ----
        # encT shares the hTm tag: its slot is recycled once precompute ends.
        encT = hpool.tile([128, DC, bt], bf16, tag="hTm", name="encT")
        nc.sync.dma_start(out=encT, in_=enc_d[:].rearrange("(k p) b -> p k b", p=128))
        cT = {}
        hT = {}
        for L, (h0s, c0s) in {"m": (h0_d, c0_d), "y": (h0_d, c0_d)}.items():
            cT[L] = spool.tile([128, DC, bt], f32, tag=f"cT{L}", name=f"cT{L}")
            nc.sync.dma_start(
                out=cT[L], in_=c0s[:].rearrange("(k p) b -> p k b", p=128)
            )
            hT[L] = hpool.tile([128, DC, bt], bf16, tag=f"hT{L}", name=f"hT{L}0")
            nc.sync.dma_start(
                out=hT[L], in_=h0s[:].rearrange("(k p) b -> p k b", p=128)
            )

        # ---- precompute xW = enc @ W_e + b  ->  DRAM scratch (bf16) ----
        for L, (we_d, b_t, xw_d) in {
            "m": (wem_d, bm_t, xwm_d),
            "y": (wey_d, by_t, xwy_d),
        }.items():
            for m in range(GC):
                ps = zpsum.tile([128, bt], f32, tag="z")
                for k in range(DC):
                    we_km = prep.tile([128, 128], bf16, tag="wek", name="wekm")
                    nc.sync.dma_start(
                        out=we_km,
                        in_=we_d[:][
                            k * 128 : (k + 1) * 128, m * 128 : (m + 1) * 128
                        ],
                    )
                    for n in range(nb):
                        sl = slice(n * NB, (n + 1) * NB)
                        nc.tensor.matmul(
                            ps[:, sl],
                            we_km,
                            encT[:, k, sl],
                            start=(k == 0),
                            stop=(k == DC - 1),
                        )
                xz = zsbp.tile([128, bt], bf16, tag="xz")
                nc.scalar.activation(
                    xz, ps, AF.Identity, bias=b_t[:, m : m + 1], scale=1.0
                )
                nc.sync.dma_start(out=xw_d[:][m], in_=xz)

        # ---- recurrent steps ----
        # (act_func, psum/stage row range, bias rows) per head, 32-aligned.
        head_groups = {
            "m": [(AF.Exp, 0, 15), (AF.Tanh, 32, 37), (AF.Identity, 64, 74)],
            "y": [(AF.Exp, 0, 10), (AF.Identity, 32, 37)],
        }
        # (stage row range -> dram out row offset) DMA blocks per head.
        out_blocks = {
            "m": [(0, 15, 0), (32, 37, 15), (64, 74, 20)],
            "y": [(0, 10, 0), (32, 37, 10)],
        }
        hdims = {"m": 96, "y": 64}
        u_ts = {"m": um_t, "y": uy_t}
        wc_ts = {"m": wcm_t, "y": wcy_t}
        wh_ts = {"m": whm_t, "y": why_t}
        xw_ds = {"m": xwm_d, "y": xwy_d}
        btabs = {"m": btm_t, "y": bty_t}
        om_ds = {"m": om_d, "y": oy_d}

        # Gate f's PSUM evacuation rides on ACT (which has headroom) so the
        # xW add runs in DVE's fast 2x SBUF mode — balances DVE down toward
        # PE's busy level (cost model: 4.285 -> 4.05 ms per pass).
        zsplit_gates = (0,)
        for tok in ablate:
            if tok.startswith("zs"):
                zsplit_gates = tuple(int(c) for c in tok[2:])
        for s in range(n_steps):
            condt = spool.tile([3, bt], bf16, tag="cond", bufs=4, name=f"cond{s}")
            nc.sync.dma_start(out=condt, in_=cond_d[:][:, s, :])
            for L in ("m", "y"):
                u_t, wc_t, wh_t = u_ts[L], wc_ts[L], wh_ts[L]
                xw_d, btab, om_dl = xw_ds[L], btabs[L], om_ds[L]
                hT_old = hT[L]
                hT_new = hpool.tile(
                    [128, DC, bt], mybir.dt.bfloat16, tag=f"hT{L}", name=f"hT{L}s{s}"
                )
                for dc in range(DC):
                    tg = {}
                    for g in range(4):
                        m = g * DC + dc
                        if "bfps" in ablate:
                            ps = zpsum.tile(
                                [128, bt], mybir.dt.bfloat16, tag="zb", name="psb"
                            )
                        else:
                            ps = zpsum.tile([128, bt], f32, tag="z")
                        for n in range(nb):
                            sl = slice(n * NB, (n + 1) * NB)
                            if "cond" not in ablate:
                                nc.tensor.matmul(
                                    ps[:, sl],
                                    wc_t[:, m * 128 : (m + 1) * 128],
                                    condt[:, sl],
                                    start=True,
                                    stop=False,
                                )
                            for k in range(DC):
                                nc.tensor.matmul(
                                    ps[:, sl],
                                    u_t[:, k, m * 128 : (m + 1) * 128],
                                    hT_old[:, k, sl],
                                    start=("cond" in ablate and k == 0),
                                    stop=(k == DC - 1),
                                )
                        tgt = tgp.tile([128, bt], mybir.dt.bfloat16, tag=f"tg{g}")
                        if "zadd" in ablate:
                            nc.scalar.activation(
                                tgt, ps, AF.Tanh, scale=(1.0 if g == 2 else 0.5)
                            )
                        else:
                            xz = xwin.tile([128, bt], mybir.dt.bfloat16, tag="xzin")
                            nc.sync.dma_start(out=xz, in_=xw_d[:][m])
                            zsb = zsbp.tile([128, bt], mybir.dt.bfloat16, tag="zsb")
                            if g in zsplit_gates:
                                # Balance engines: ACT evacuates PSUM (bf16),
                                # DVE adds xW at its fast 2x SBUF mode.
                                zc = zsbp.tile(
                                    [128, bt], mybir.dt.bfloat16, tag="zc",
                                    name="zc",
                                )
                                nc.scalar.activation(zc, ps, AF.Copy)
                                nc.vector.tensor_add(zsb, zc, xz)
                            else:
                                nc.vector.tensor_add(zsb, ps, xz)
                            nc.scalar.activation(
                                tgt, zsb, AF.Tanh, scale=(1.0 if g == 2 else 0.5)
                            )
                        tg[g] = tgt
                    if "elem" in ablate:
                        continue
                    # elementwise: c' = sig(f)c + sig(i)tanh(g); h' = sig(o)tanh(c')
                    A = ep.tile([128, bt], f32, tag="A")
                    nc.vector.scalar_tensor_tensor(
                        A, tg[1], 1.0, cT[L][:, dc, :], ALU.add, ALU.mult
                    )
                    si = ep.tile([128, bt], mybir.dt.bfloat16, tag="si")
                    if "sidve" in ablate:
                        nc.vector.tensor_scalar(
                            si, tg[0], 0.5, 0.5, ALU.mult, ALU.add
                        )
                    else:
                        nc.scalar.activation(si, tg[0], AF.Copy, bias=0.5, scale=0.5)
                    Bt = ep.tile([128, bt], mybir.dt.bfloat16, tag="Bt")
                    nc.vector.tensor_mul(Bt, si, tg[2])
                    nc.vector.scalar_tensor_tensor(
                        cT[L][:, dc, :], A, 0.5, Bt, ALU.mult, ALU.add
                    )
                    tc2 = ep.tile([128, bt], mybir.dt.bfloat16, tag="tc2")
                    nc.scalar.activation(tc2, cT[L][:, dc, :], AF.Tanh)
                    nc.vector.scalar_tensor_tensor(
                        hT_new[:, dc, :], tg[3], 1.0, tc2, ALU.add, ALU.mult
                    )
                if "norec" not in ablate:
                    hT[L] = hT_new
                if "heads" in ablate:
                    continue
                # ---- MDN head ----
                hd = hdims[L]
                omt = omp.tile([hd, bt], mybir.dt.bfloat16, tag=f"om{L}")
                for n in range(nb):
                    sl = slice(n * NB, (n + 1) * NB)
                    hp = hpsum.tile([hd, NB], f32, tag="hp")
                    for k in range(DC):
                        nc.tensor.matmul(
                            hp,
                            wh_t[:, k, :],
                            hT_new[:, k, sl],
                            start=(k == 0),
                            stop=(k == DC - 1),
                        )
                    for (fn, a, b) in head_groups[L]:
                        nc.scalar.activation(
                            omt[a:b, sl], hp[a:b, :], fn,
                            bias=btab[a:b, s : s + 1],
                        )
                    sp = spsum.tile([5, NB], f32, tag="sp")
                    nc.tensor.matmul(sp, ones5, omt[0:5, sl], start=True, stop=True)
                    rc = ep.tile([5, NB], f32, tag="rc")
                    nc.vector.reciprocal(rc, sp)
                    nc.vector.tensor_mul(omt[0:5, sl], omt[0:5, sl], rc)
                for (a, b, o) in out_blocks[L]:
                    nc.sync.dma_start(
                        out=om_dl[:][s, o : o + (b - a), :], in_=omt[a:b, :]
                    )

    nc.compile()
    return nc


# Device head layout: (device col base, reference col slice) pairs.
#   merger ref cols: a=0:5 ml=5:10 sl=10:15 mlt=15:20 slt=20:25 r=25:30
HEAD_COLS_M = [(0, 0, 5), (5, 10, 15), (10, 20, 25), (32, 25, 30),
               (64, 5, 10), (69, 15, 20)]
#   yielder ref cols: a=0:5 ml=5:10 sl=10:15
HEAD_COLS_Y = [(0, 0, 5), (5, 10, 15), (32, 5, 10)]
# Device out rows -> reference out rows (om: [alpha, esl, eslt, tanh_r, ml, mlt])
PERM_M = np.array(
    list(range(0, 5)) + list(range(20, 25)) + list(range(5, 10))
    + list(range(25, 30)) + list(range(10, 15)) + list(range(15, 20))
)
PERM_Y = np.array(list(range(0, 5)) + list(range(10, 15)) + list(range(5, 10)))


def _pad_head(w, cols, width):
    """w [rows, ref_cols] -> padded [rows, width] in device column layout."""
    out = np.zeros((w.shape[0], width), w.dtype)
    for dev0, r0, r1 in cols:
        out[:, dev0 : dev0 + (r1 - r0)] = w[:, r0:r1]
    return out


def prep_host_inputs(inputs, n_steps=S):
    """Full-batch host-side prep. Returns dict of full-width arrays."""
    Wm = np.asarray(inputs["W_m"], F32)
    Wy = np.asarray(inputs["W_y"], F32)
    Um = np.asarray(inputs["U_m"], F32)
    Uy = np.asarray(inputs["U_y"], F32)
    Whm = np.asarray(inputs["Wh_m"], F32)
    Why = np.asarray(inputs["Wh_y"], F32)
    bm = np.asarray(inputs["b_m"], F32)
    by = np.asarray(inputs["b_y"], F32)
    bhm = np.asarray(inputs["bh_m"], F32)
    bhy = np.asarray(inputs["bh_y"], F32)
    sh = np.asarray(inputs["state_h"], F32)
    sc = np.asarray(inputs["state_c"], F32)
    cond = np.asarray(inputs["conditions"], F32)

    shT = np.ascontiguousarray(sh.T)                      # [512, B]
    out = {
        "um": np.ascontiguousarray((Um * 0.5).astype(BF16)),
        "uy": np.ascontiguousarray((Uy * 0.5).astype(BF16)),
        "wem": np.ascontiguousarray(Wm[:D].astype(BF16)),
        "wey": np.ascontiguousarray(Wy[:D].astype(BF16)),
        "wcm": np.ascontiguousarray(Wm[D:].astype(BF16)),
        "wcy": np.ascontiguousarray(Wy[D:].astype(BF16)),
        "whm": _pad_head(Whm[:D] * 0.5, HEAD_COLS_M, 96).astype(BF16),
        "why": _pad_head(Why[:D] * 0.5, HEAD_COLS_Y, 64).astype(BF16),
        "bm": np.ascontiguousarray(bm.reshape(GC, 128).T.astype(F32)),
        "by": np.ascontiguousarray(by.reshape(GC, 128).T.astype(F32)),
        "btm": _pad_head(
            (bhm[None, :] + np.cumsum(Whm[D : D + n_steps], 0)), HEAD_COLS_M, 96
        ).T.astype(F32).copy(),
        "bty": _pad_head(
            (bhy[None, :] + np.cumsum(Why[D : D + n_steps], 0)), HEAD_COLS_Y, 64
        ).T.astype(F32).copy(),
        "encT": np.ascontiguousarray(shT.astype(BF16)),   # [512, B]
        "h0T": np.ascontiguousarray((2.0 * shT).astype(BF16)),
        "c0T": np.ascontiguousarray(sc.T.astype(F32)),    # [512, B]
        "condT": np.ascontiguousarray(
            cond.transpose(2, 1, 0)[:, :n_steps, :].astype(BF16)
        ),                                                # [3, S, B]
    }
    return out


def make_in_maps(hp, col0, bt, n_cores=NCORES, stride=BPC):
    """One in_map per core for a pass covering columns [core*stride+col0, +bt)."""
    shared_keys = ["um", "uy", "wem", "wey", "wcm", "wcy", "whm", "why",
                   "bm", "by", "btm", "bty"]
    maps = []
    for ci in range(n_cores):
        a = ci * stride + col0
        sl = slice(a, a + bt)
        m = {k: hp[k] for k in shared_keys}
        m["enc"] = np.ascontiguousarray(hp["encT"][:, sl])
        m["h0"] = np.ascontiguousarray(hp["h0T"][:, sl])
        m["c0"] = np.ascontiguousarray(hp["c0T"][:, sl])
        m["cond"] = np.ascontiguousarray(hp["condT"][:, :, sl])
        maps.append(m)
    return maps


_PROG = {}


def _get_program(n_steps=S, bt=BT):
    key = (n_steps, bt)
    if key not in _PROG:
        _PROG[key] = build_program(n_steps, bt)
    return _PROG[key]


def kernel(**inputs):
    nc = _get_program()
    hp = prep_host_inputs(inputs)
    om_all = np.empty((S, 30, B), BF16)
    oy_all = np.empty((S, 15, B), BF16)
    for pi in range(BPC // BT):
        maps = make_in_maps(hp, pi * BT, BT)
        res = run_bass_kernel_spmd(nc, maps, core_ids=list(range(NCORES)))
        for ci in range(NCORES):
            a = ci * BPC + pi * BT
            sl = slice(a, a + BT)
            om_all[:, :, sl] = res.results[ci]["om"]
            oy_all[:, :, sl] = res.results[ci]["oy"]
    param_m = np.ascontiguousarray(
        om_all.transpose(2, 0, 1)[:, :, PERM_M].astype(F32)
    )
    param_y = np.ascontiguousarray(
        oy_all.transpose(2, 0, 1)[:, :, PERM_Y].astype(F32)
    )
    return param_m, param_y
